# revision 1
# baseline (speedup 1.0000x reference)
"""3-layer GraphSAGE (mean aggregator) + classifier on 8 Trainium2 NeuronCores.

Strategy (dst-node sharding):
  - Nodes padded to NPAD=50176 and split into 8 shards of 6272 (49 tiles of 128).
  - Host sorts edges by (core, dst-tile, src<32768), chunks them into groups of
    <=128 edges per (tile, half).  Chunk counts are maxed across cores so all 8
    cores run one identical program (SPMD); shorter cores pad with idx=0 /
    dstloc=-1 edges which contribute exactly zero.
  - Each layer: dma_gather fetches h[src] rows (fp16, 256B) edge-major into
    SBUF; a one-hot selector S (built on DVE from dstloc via iota==dstloc with
    0-stride broadcast APs) turns segment-sum into PE matmuls accumulated in
    PSUM, giving h_neigh^T (dim-major) per 128-node tile.  inv_deg is applied
    during the PSUM->SBUF copy (tensor_tensor mult with a replicated table).
  - Dense part: h_next = relu(h@Wself + h_neigh@Wneigh + b) as three PE
    matmuls per tile (bias via a K=1 matmul with a ones row).  Layers 0..L-2
    produce node-major output tiles which are DMAed to a DRAM bounce buffer
    and AllGathered to every core's full-h buffer (the next layer's gather
    source); the last layer produces dim-major h^T kept in SBUF.
  - Classifier + softmax per tile, node-major, written to the output shard.
"""

import os
import sys

for _p in ("/opt/trn_rl_repo", "/root/.axon_site/_ro/trn_rl_repo"):
    if os.path.isdir(_p) and _p not in sys.path:
        sys.path.insert(0, _p)

import numpy as np

import concourse.bass as bass
import concourse.bacc as bacc
import concourse.tile as tile
import concourse.mybir as mybir

F16 = mybir.dt.float16
F32 = mybir.dt.float32
I16 = mybir.dt.int16
TILE = 128


def _ceil_div(a, b):
    return -(-a // b)


def _wrap_idx(a):
    """[n] int16 -> [128, n//16]: idx i at partition i%16 col i//16, x8 replicated."""
    n = a.shape[0]
    w = a.reshape(n // 16, 16).T
    return np.tile(w, (8, 1)).astype(np.int16)


def preprocess(src, dst, N, cfg):
    """Host-side graph preprocessing -> per-core input arrays + static schedule."""
    NC, NPAD, LO, GT, L = cfg["NC"], cfg["NPAD"], cfg["LO"], cfg["GT"], cfg["L"]
    SHARD = NPAD // NC
    TPC = SHARD // TILE
    E = src.shape[0]

    src = src.astype(np.int64)
    dst = dst.astype(np.int64)
    core = dst // SHARD
    loc = dst % SHARD
    tl = loc // TILE
    jj = loc % TILE
    half = (src >= LO).astype(np.int64)

    key = (core * TPC + tl) * 2 + half
    cnt = np.bincount(key, minlength=NC * TPC * 2)
    cnt3 = cnt.reshape(NC, TPC, 2)
    NL = _ceil_div(cnt3[:, :, 0], TILE).max(axis=0)  # [TPC]
    NH = _ceil_div(cnt3[:, :, 1], TILE).max(axis=0)
    OFFLO = np.concatenate([[0], np.cumsum(NL)])  # chunk offsets per tile
    OFFHI = np.concatenate([[0], np.cumsum(NH)])
    TOTLO, TOTHI = int(OFFLO[-1]), int(OFFHI[-1])

    NG = _ceil_div(TPC, GT)
    groups = [(g * GT, min((g + 1) * GT, TPC)) for g in range(NG)]
    # call chunk ranges per group
    calls_lo = [(int(OFFLO[a]), int(OFFLO[b])) for a, b in groups]
    calls_hi = [(int(OFFHI[a]), int(OFFHI[b])) for a, b in groups]

    # rank of each edge within its (core,tile,half) bucket
    order = np.argsort(key, kind="stable")
    starts = np.concatenate([[0], np.cumsum(cnt)])[:-1]
    rank = np.arange(E) - np.repeat(starts, cnt)
    e_src, e_core, e_tl, e_j, e_half = (
        src[order], core[order], tl[order], jj[order], half[order])

    deg = np.bincount(dst, minlength=N).astype(np.float32)
    ideg = 1.0 / np.maximum(deg, 1.0)
    ideg_pad = np.ones(NPAD, np.float32)
    ideg_pad[:N] = ideg

    per_core = []
    for c in range(NC):
        sel_lo = (e_core == c) & (e_half == 0)
        sel_hi = (e_core == c) & (e_half == 1)
        pos_lo = OFFLO[e_tl[sel_lo]] * TILE + rank[sel_lo]
        pos_hi = OFFHI[e_tl[sel_hi]] * TILE + rank[sel_hi]

        idx_lo = np.zeros(max(TOTLO, 1) * TILE, np.int16)
        dl_lo = np.full(max(TOTLO, 1) * TILE, -1.0, np.float16)
        idx_lo[pos_lo] = e_src[sel_lo]
        dl_lo[pos_lo] = e_j[sel_lo]
        idx_hi = np.zeros(max(TOTHI, 1) * TILE, np.int16)
        dl_hi = np.full(max(TOTHI, 1) * TILE, -1.0, np.float16)
        idx_hi[pos_hi] = (e_src[sel_hi] - LO)
        dl_hi[pos_hi] = e_j[sel_hi]

        gidx_lo = _wrap_idx(idx_lo)
        gidx_hi = _wrap_idx(idx_hi)

        per_core.append({
            "gidx_lo": gidx_lo,
            "gidx_hi": gidx_hi,
            "dstloc_lo": dl_lo.reshape(max(TOTLO, 1), TILE).T.copy(),
            "dstloc_hi": dl_hi.reshape(max(TOTHI, 1), TILE).T.copy(),
            "idegrep": np.tile(ideg_pad[c * SHARD:(c + 1) * SHARD].astype(np.float16), (128, 1)),
        })

    meta = {
        "NL": NL.astype(int).tolist(), "NH": NH.astype(int).tolist(),
        "OFFLO": OFFLO.astype(int).tolist(), "OFFHI": OFFHI.astype(int).tolist(),
        "TOTLO": TOTLO, "TOTHI": TOTHI,
        "groups": groups, "calls_lo": calls_lo, "calls_hi": calls_hi,
        "SHARD": SHARD, "TPC": TPC, "NG": NG,
    }
    return per_core, meta


def build_nc(cfg, meta):
    import os as _os
    SKIP = set(_os.environ.get("KERNEL_SKIP", "").split(","))
    NC, NPAD, LO, L, D, C = (cfg["NC"], cfg["NPAD"], cfg["LO"], cfg["L"],
                             cfg["D"], cfg["C"])
    SHARD, TPC = meta["SHARD"], meta["TPC"]
    NL, NH, OFFLO, OFFHI = meta["NL"], meta["NH"], meta["OFFLO"], meta["OFFHI"]
    TOTLO, TOTHI = max(meta["TOTLO"], 1), max(meta["TOTHI"], 1)
    groups, calls_lo, calls_hi = meta["groups"], meta["calls_lo"], meta["calls_hi"]
    MAXLO = max((b - a) for a, b in calls_lo) if calls_lo else 1
    MAXHI = max((b - a) for a, b in calls_hi) if calls_hi else 1
    MAXLO, MAXHI = max(MAXLO, 1), max(MAXHI, 1)

    nc = bacc.Bacc("TRN2", target_bir_lowering=False, debug=False, num_devices=NC,
                   num_swdge_queues=4)
    # dma_gather with single_packet=True is limited to 64 data descriptors per
    # SDMA lane = 1024 indices (8 chunks of 128) per call.
    CALL_CHUNKS = 8
    qrot = [0]

    def gather_calls(nc_, out_tile, in_ap, gidx_sb, c0, c1):
        for cs in range(c0, c1, CALL_CHUNKS):
            n = min(CALL_CHUNKS, c1 - cs)
            nc_.gpsimd.dma_gather(
                out_ap=out_tile[:, cs - c0:cs - c0 + n, :],
                in_ap=in_ap,
                idxs_ap=gidx_sb[:, cs * 8:(cs + n) * 8],
                num_idxs=n * TILE, num_idxs_reg=n * TILE,
                elem_size=128,
                queue_num=qrot[0] % 4,
            )
            qrot[0] += 1

    feat_own = nc.dram_tensor("feat_own", [SHARD, D], F16, kind="ExternalInput")
    gidx_lo_d = nc.dram_tensor("gidx_lo", [128, TOTLO * 8], I16, kind="ExternalInput")
    gidx_hi_d = nc.dram_tensor("gidx_hi", [128, TOTHI * 8], I16, kind="ExternalInput")
    dstloc_lo_d = nc.dram_tensor("dstloc_lo", [128, TOTLO], F16, kind="ExternalInput")
    dstloc_hi_d = nc.dram_tensor("dstloc_hi", [128, TOTHI], F16, kind="ExternalInput")
    idegrep_d = nc.dram_tensor("idegrep", [128, SHARD], F16, kind="ExternalInput")
    wself_d = nc.dram_tensor("wself", [L, D, D], F16, kind="ExternalInput")
    wneigh_d = nc.dram_tensor("wneigh", [L, D, D], F16, kind="ExternalInput")
    brow_d = nc.dram_tensor("brow", [L, 1, D], F16, kind="ExternalInput")
    wc_d = nc.dram_tensor("wc", [D, C], F16, kind="ExternalInput")
    bc_d = nc.dram_tensor("bc", [1, C], F16, kind="ExternalInput")
    out_d = nc.dram_tensor("out", [SHARD, C], F32, kind="ExternalOutput")

    with tile.TileContext(nc) as tc:
        with (
            tc.tile_pool(name="const", bufs=1) as cpool,
            tc.tile_pool(name="gbuf", bufs=2) as gpool,
            tc.tile_pool(name="spool", bufs=2) as spool,
            tc.tile_pool(name="hn", bufs=3) as hnpool,
            tc.tile_pool(name="hown", bufs=2) as hopool,
            tc.tile_pool(name="hstage", bufs=2) as hspool,
            tc.tile_pool(name="misc", bufs=2) as mpool,
            tc.tile_pool(name="ps_agg", bufs=2, space="PSUM") as ps_agg,
            tc.tile_pool(name="ps_dense", bufs=2, space="PSUM") as ps_dense,
            tc.tile_pool(name="ps_cls", bufs=2, space="PSUM") as ps_cls,
            tc.tile_pool(name="dram", bufs=1, space="DRAM") as dpool,
        ):
            # ---- constants into SBUF
            gidx_lo = cpool.tile([128, TOTLO * 8], I16)
            nc.sync.dma_start(gidx_lo[:], gidx_lo_d[:])
            gidx_hi = cpool.tile([128, TOTHI * 8], I16)
            nc.sync.dma_start(gidx_hi[:], gidx_hi_d[:])
            dstloc_lo = cpool.tile([128, TOTLO], F16)
            nc.sync.dma_start(dstloc_lo[:], dstloc_lo_d[:])
            dstloc_hi = cpool.tile([128, TOTHI], F16)
            nc.sync.dma_start(dstloc_hi[:], dstloc_hi_d[:])
            idegrep = cpool.tile([128, SHARD], F16)
            nc.sync.dma_start(idegrep[:], idegrep_d[:])
            wself = cpool.tile([128, L, D], F16)
            nc.sync.dma_start(wself[:], wself_d.rearrange("l k n -> k l n"))
            wneigh = cpool.tile([128, L, D], F16)
            nc.sync.dma_start(wneigh[:], wneigh_d.rearrange("l k n -> k l n"))
            brow = cpool.tile([1, L, D], F16)
            nc.sync.dma_start(brow[:], brow_d.rearrange("l o n -> o l n"))
            wc = cpool.tile([128, C], F16)
            nc.sync.dma_start(wc[:], wc_d[:])
            bc = cpool.tile([1, C], F16)
            nc.sync.dma_start(bc[:], bc_d[:])
            iota = cpool.tile([128, 128], F16)
            nc.gpsimd.iota(iota[:], pattern=[[1, 128]], base=0, channel_multiplier=0,
                           allow_small_or_imprecise_dtypes=True)
            ones_row = cpool.tile([1, 128], F16)
            nc.vector.memset(ones_row[:], 1.0)

            # ---- initial AllGather of fp16 features
            binit = dpool.tile([SHARD, D], F16)
            nc.sync.dma_start(binit[:], feat_own[:])
            shared = "Shared" if NC > 4 else "Local"
            hbuf0 = dpool.tile([NPAD, D], F16, addr_space=shared)
            if "ag" not in SKIP:
                nc.gpsimd.collective_compute(
                    "AllGather", mybir.AluOpType.bypass,
                    replica_groups=[list(range(NC))],
                    ins=[binit[:].opt()], outs=[hbuf0[:].opt()],
                )

            hbuf = hbuf0
            hOwnT = hopool.tile([128, SHARD], F16, tag="hOwnT")
            nc.sync.dma_start_transpose(hOwnT[:], feat_own[:])
            h3T = None

            for l in range(L):
                last = l == L - 1
                if last:
                    h3T = cpool.tile([128, SHARD], F16)
                else:
                    hstage = hspool.tile([128, TPC, D], F16, tag="hstage")

                for gi, (t0, t1) in enumerate(groups):
                    clo0, clo1 = calls_lo[gi]
                    chi0, chi1 = calls_hi[gi]
                    nlo, nhi = clo1 - clo0, chi1 - chi0
                    glo = gpool.tile([128, MAXLO, D], F16, tag="glo")
                    if nlo and "gather" not in SKIP:
                        gather_calls(nc, glo, hbuf[0:LO, :], gidx_lo, clo0, clo1)
                    ghi = gpool.tile([128, MAXHI, D], F16, tag="ghi")
                    if nhi and "gather" not in SKIP:
                        gather_calls(nc, ghi, hbuf[LO:NPAD, :], gidx_hi, chi0, chi1)
                    slo = spool.tile([128, MAXLO, 128], F16, tag="slo")
                    if nlo and "sbuild" not in SKIP:
                        nc.vector.tensor_tensor(
                            slo[:, 0:nlo, :],
                            iota[:].unsqueeze(1).broadcast_to([128, nlo, 128]),
                            dstloc_lo[:, clo0:clo1].unsqueeze(2).broadcast_to([128, nlo, 128]),
                            mybir.AluOpType.is_equal,
                        )
                    shi = spool.tile([128, MAXHI, 128], F16, tag="shi")
                    if nhi and "sbuild" not in SKIP:
                        nc.vector.tensor_tensor(
                            shi[:, 0:nhi, :],
                            iota[:].unsqueeze(1).broadcast_to([128, nhi, 128]),
                            dstloc_hi[:, chi0:chi1].unsqueeze(2).broadcast_to([128, nhi, 128]),
                            mybir.AluOpType.is_equal,
                        )

                    for t in range(t0, t1):
                        ntot = NL[t] + NH[t]
                        hneighT = hnpool.tile([128, 128], F16, tag="hneighT")
                        if ntot == 0 or "agg" in SKIP:
                            nc.vector.memset(hneighT[:], 0.0)
                        elif True:
                            agg = ps_agg.tile([128, 128], F32)
                            k = 0
                            for q in range(NL[t]):
                                s = OFFLO[t] - clo0 + q
                                nc.tensor.matmul(agg[:], glo[:, s, :], slo[:, s, :],
                                                 start=(k == 0), stop=(k == ntot - 1))
                                k += 1
                            for q in range(NH[t]):
                                s = OFFHI[t] - chi0 + q
                                nc.tensor.matmul(agg[:], ghi[:, s, :], shi[:, s, :],
                                                 start=(k == 0), stop=(k == ntot - 1))
                                k += 1
                            # scale by inv_deg while copying PSUM -> SBUF fp16
                            nc.vector.tensor_tensor(
                                hneighT[:], agg[:],
                                idegrep[:, t * 128:(t + 1) * 128],
                                mybir.AluOpType.mult,
                            )

                        ts = slice(t * 128, (t + 1) * 128)
                        if not last:
                            pd = ps_dense.tile([128, 128], F32)
                            nc.tensor.matmul(pd[:], hOwnT[:, ts], wself[:, l, :],
                                             start=True, stop=False)
                            nc.tensor.matmul(pd[:], hneighT[:], wneigh[:, l, :],
                                             start=False, stop=False)
                            nc.tensor.matmul(pd[:], ones_row[:], brow[:, l, :],
                                             start=False, stop=True)
                            nc.scalar.activation(hstage[:, t, :], pd[:],
                                                 mybir.ActivationFunctionType.Relu)
                        else:
                            pd = ps_dense.tile([128, 128], F32)
                            nc.tensor.matmul(pd[:], wself[:, l, :], hOwnT[:, ts],
                                             start=True, stop=False)
                            nc.tensor.matmul(pd[:], wneigh[:, l, :], hneighT[:],
                                             start=False, stop=False)
                            nc.tensor.matmul(pd[:], brow[:, l, :], ones_row[:],
                                             start=False, stop=True)
                            nc.scalar.activation(h3T[:, ts], pd[:],
                                                 mybir.ActivationFunctionType.Relu)

                if not last:
                    bounce = dpool.tile([SHARD, D], F16, tag="bounce", bufs=2)
                    nc.sync.dma_start(
                        bounce.rearrange("(t p) d -> p t d", p=128), hstage[:])
                    hbuf = dpool.tile([NPAD, D], F16, addr_space=shared,
                                      tag="hbuf", bufs=2)
                    if "ag" not in SKIP:
                        nc.gpsimd.collective_compute(
                            "AllGather", mybir.AluOpType.bypass,
                            replica_groups=[list(range(NC))],
                            ins=[bounce[:].opt()], outs=[hbuf[:].opt()],
                        )
                    hOwnT = hopool.tile([128, SHARD], F16, tag="hOwnT")
                    nc.sync.dma_start_transpose(hOwnT[:], bounce[:])

            # ---- classifier + softmax (node-major per tile)
            out_stage = cpool.tile([128, TPC, C], F32)
            for t in range(TPC):
                ts = slice(t * 128, (t + 1) * 128)
                pc = ps_cls.tile([128, C], F32)
                nc.tensor.matmul(pc[:], h3T[:, ts], wc[:], start=True, stop=False)
                nc.tensor.matmul(pc[:], ones_row[:], bc[:], start=False, stop=True)
                mx = mpool.tile([128, 1], F32, tag="mx")
                nc.vector.reduce_max(mx[:], pc[:], mybir.AxisListType.X, negate=True)
                ex = mpool.tile([128, C], F32, tag="ex")
                nc.scalar.activation(ex[:], pc[:], mybir.ActivationFunctionType.Exp,
                                     bias=mx[:])
                sm = mpool.tile([128, 1], F32, tag="sm")
                nc.vector.reduce_sum(sm[:], ex[:], mybir.AxisListType.X)
                rc = mpool.tile([128, 1], F32, tag="rc")
                nc.vector.reciprocal(rc[:], sm[:])
                nc.vector.tensor_scalar(out_stage[:, t, :], ex[:], rc[:], None,
                                        mybir.AluOpType.mult)
            nc.sync.dma_start(out_d.rearrange("(t p) c -> p t c", p=128), out_stage[:])

    nc.compile()
    return nc


def make_inputs(features, w_self, w_neigh, b, wc, bc, per_core, cfg, meta):
    NC, NPAD = cfg["NC"], cfg["NPAD"]
    SHARD = meta["SHARD"]
    N = features.shape[0]
    feat_pad = np.zeros((NPAD, cfg["D"]), np.float16)
    feat_pad[:N] = features.astype(np.float16)
    in_maps = []
    for c in range(NC):
        m = dict(per_core[c])
        m["feat_own"] = feat_pad[c * SHARD:(c + 1) * SHARD]
        m["wself"] = w_self.astype(np.float16)
        m["wneigh"] = w_neigh.astype(np.float16)
        m["brow"] = b.astype(np.float16).reshape(cfg["L"], 1, cfg["D"])
        m["wc"] = wc.astype(np.float16)
        m["bc"] = bc.astype(np.float16).reshape(1, cfg["C"])
        in_maps.append(m)
    return in_maps


DEFAULT_CFG = dict(NC=8, NPAD=50176, LO=32768, GT=5, L=3, D=128, C=47)

_CACHE = {}


def kernel(features, src, dst, w_self, w_neigh, b, wc, bc):
    from concourse import bass_utils

    cfg = DEFAULT_CFG
    N = features.shape[0]
    key = (hash(src.tobytes()), hash(dst.tobytes()), N)
    if key not in _CACHE:
        per_core, meta = preprocess(np.asarray(src), np.asarray(dst), N, cfg)
        nc = build_nc(cfg, meta)
        _CACHE[key] = (per_core, meta, nc)
    per_core, meta, nc = _CACHE[key]

    in_maps = make_inputs(np.asarray(features), np.asarray(w_self),
                          np.asarray(w_neigh), np.asarray(b), np.asarray(wc),
                          np.asarray(bc), per_core, cfg, meta)
    res = bass_utils.run_bass_kernel_spmd(nc, in_maps, core_ids=list(range(cfg["NC"])))
    out = np.concatenate([res.results[c]["out"] for c in range(cfg["NC"])], axis=0)
    return out[:N].astype(np.float32)

